# revision 23
# baseline (speedup 1.0000x reference)
"""KoLeo-loss kernel for Trainium2, 8 NeuronCores — symmetric Gram + fp8.

Math: rows are L2-normalized; for unit vectors dist(a,b) = sqrt(2-2*a.b), so
the per-row NN distance needs only the row-max of the diagonal-masked cosine
Gram matrix.  G = Y Y^T is symmetric: each off-diagonal 512x512 group-block is
computed ONCE and yields row-max partials (free-dim reduce on DVE) plus
col-max partials (partition-dim reduce via GPSIMD partition_all_reduce).
This halves matmul FLOPs and HBM traffic vs computing full Gram rows.

Precision: Y is quantized to fp8e4 (x16 scale; dots then x256) and the Gram
runs as DoubleRow fp8 matmuls — 4 matmuls of K=256 per [128,512] block, 2x
the bf16 rate.  fp8 rounding moves the loss ~1e-3 relative (threshold 2e-2).
Layout: fp8 y is transposed as packed f32 words (4 fp8/word) giving
YT[p, v, q, row-word]; word-block q in {0,1} spans d = 4*(128q+p)+t.
DoubleRow k-subtile pairs run over q (plane stride - ISA-encodable, verified
on HW); the 4 K-phases run over byte index t.  Only 4 PE transposes/chunk.

Work split (8 groups of 512 rows): 28 off-diag group pairs + 8 diag
triangles.  Core k gets groups (r0, r1, c0, c1): units u01=(r0 x c0),
u31=(r1 x c0), u32=(r1 x c1), u20=(c1 x r0) [that pair computed transposed
so it row-streams off c1's chunks], plus the masked diagonal triangle of r0.
All 28 pairs + 8 diagonals covered (4 pairs twice — harmless for max).
Every core runs the IDENTICAL program; the host packs xg in processing
order [c0, r0, c1, r1] and maps the partial maxes back.

Pipelining: the front-end is emitted in two stages with a one-chunk skew
(A: dma + squares + rsqrt chain, B: fp8 stores + transposes + copyback +
dependent Gram blocks) so each in-order engine queue never head-of-line
blocks on another engine's freshly-issued work.  Engine balance per chunk:
ACT = square(v0) + sqrt + both fp8 scale-stores; DVE = square(v1) via
scalar_tensor_tensor-accum + reciprocal + copyback + all PSUM drains
(row reduce_max + col max-merge); GPSIMD = partition_all_reduce only.
"""

import os
import sys
from contextlib import ExitStack

import numpy as np

sys.path.insert(0, "/opt/trn_rl_repo")

import concourse.bass as bass
import concourse.mybir as mybir
import concourse.tile as tile
from concourse import bacc, bass_isa, bass_utils

F32 = mybir.dt.float32
F8 = mybir.dt.float8e4
F16 = mybir.dt.float16
BF16 = mybir.dt.bfloat16
AF = mybir.ActivationFunctionType
ALU = mybir.AluOpType
PM = mybir.MatmulPerfMode

B, V, D = 4096, 2, 1024
NCORES = 8
G, GS, NCH = 8, 512, 4
EPS = 1e-8
YSC = 16.0          # fp8 quantization scale for y; dots scale by YSC^2
MASKV = -1024.0     # below -YSC^2, dominated by any real dot

# per-core groups (r0, r1, c0, c1); r0 also carries the diagonal triangle.
CORES = [
    (0, 1, 2, 3),
    (1, 0, 4, 5),
    (6, 7, 0, 1),
    (2, 3, 4, 5),
    (3, 2, 6, 7),
    (4, 5, 6, 7),
    (5, 0, 1, 4),
    (7, 2, 3, 6),
]
# xg slot packing (= load/processing order): [c0, r0, c1, r1]
S_C0, S_R0, S_C1, S_R1 = 0, 1, 2, 3


def build():
    nc = bacc.Bacc("TRN2", debug=False)
    xg_d = nc.dram_tensor("xg", [4, GS, V, D], F32, kind="ExternalInput").ap()
    cstF_d = nc.dram_tensor("cstF", [128, 129], F32, kind="ExternalInput").ap()
    cstB_d = nc.dram_tensor("cstB", [128, 256], BF16, kind="ExternalInput").ap()
    rm_d = nc.dram_tensor("rm", [128, V * 3 * NCH], F32, kind="ExternalOutput").ap()
    cm_d = nc.dram_tensor("cm", [1, V * 3 * GS], F16, kind="ExternalOutput").ap()

    with ExitStack() as ctx:
        tc = ctx.enter_context(tile.TileContext(nc))
        const = ctx.enter_context(tc.tile_pool(name="const", bufs=1))
        xpool = ctx.enter_context(tc.tile_pool(name="xpool", bufs=8))
        ypool = ctx.enter_context(tc.tile_pool(name="ypool", bufs=4))
        sqpool = ctx.enter_context(tc.tile_pool(name="sqpool", bufs=3))
        sspool = ctx.enter_context(tc.tile_pool(name="sspool", bufs=4))
        accp = ctx.enter_context(tc.tile_pool(name="accp", bufs=3, space="PSUM"))
        trp = ctx.enter_context(tc.tile_pool(name="trp", bufs=2, space="PSUM"))
        scrp = ctx.enter_context(tc.tile_pool(name="scrp", bufs=3))

        cstF = const.tile([128, 129], F32, name="cstF")
        nc.sync.dma_start(cstF[:], cstF_d)
        identF = cstF[:, 0:128]
        epsb = cstF[:, 128:129]
        cstB = const.tile([128, 256], BF16, name="cstB")
        nc.sync.dma_start(cstB[:], cstB_d)
        identB = cstB[:, 0:128]
        negI = cstB[:, 128:256]

        # persistent transposed fp8 (word-packed) buffers, one per slot
        YT = [const.tile([128, V, 2, GS], F32, name=f"YT{s}") for s in range(4)]
        YTr = [t.bitcast(F8).rearrange("p v q (j t) -> p v q t j", t=4) for t in YT]

        # rmcoll[p, v, rslot, chunk, contrib]; rslots: 0=r0, 1=r1, 2=c1
        rmcoll = const.tile([128, V, 3, NCH, 2], F32, name="rmcoll")
        nc.gpsimd.memset(rmcoll[:], MASKV)
        # cmx[p, v, cslot, :]; cslots: 0=c0, 1=c1, 2=r0 (diag + u20)
        cmx = const.tile([128, V, 3, GS], F16, name="cmx")
        nc.gpsimd.memset(cmx[:], MASKV)
        cmr = const.tile([128, V, 3, GS], F16, name="cmr")

        state = {}

        def stage_a(s, c, both_dve):
            """DMA + norms for chunk (s, c).  Squares alternate between
            ACT+DVE and DVE-only chunks to balance the two engines."""
            xt = xpool.tile([128, V, D], F32, tag="xraw", name="xraw")
            nc.sync.dma_start(xt[:], xg_d[s, 128 * c : 128 * (c + 1)])
            ss = sspool.tile([128, V], F32, tag="ss", name="ss")
            if both_dve:
                sq0 = sqpool.tile([128, D], BF16, tag="sq0", name="sq0")
                nc.vector.scalar_tensor_tensor(
                    sq0[:], xt[:, 0, :], 1.0, xt[:, 0, :], ALU.mult, ALU.mult,
                    accum_out=ss[:, 0:1])
            else:
                sq0 = sqpool.tile([128, D], BF16, tag="sq0", name="sq0")
                nc.scalar.activation(
                    sq0[:], xt[:, 0, :], AF.Square, accum_out=ss[:, 0:1])
            sq1 = sqpool.tile([128, D], BF16, tag="sq1", name="sq1")
            nc.vector.scalar_tensor_tensor(
                sq1[:], xt[:, 1, :], 1.0, xt[:, 1, :], ALU.mult, ALU.mult,
                accum_out=ss[:, 1:2])
            # nrm = sqrt(ss+eps)/YSC so that rinv = YSC/|x|
            nrm = sspool.tile([128, V], F32, tag="nrm", name="nrm")
            nc.scalar.activation(
                nrm[:], ss[:], AF.Sqrt, bias=epsb, scale=1.0 / (YSC * YSC))
            rinv = sspool.tile([128, V], F32, tag="rinv", name="rinv")
            nc.vector.reciprocal(rinv[:], nrm[:])
            state[(s, c)] = (xt, rinv)

        def stage_b(s, c):
            """fp8 stores + transposes + copyback for chunk (s, c)."""
            xt, rinv = state.pop((s, c))
            y8 = ypool.tile([128, V, D], F8, tag="y8", name="y8")
            for v in range(V):
                nc.scalar.activation(
                    y8[:, v, :], xt[:, v, :], AF.Copy, bias=0.0,
                    scale=rinv[:, v : v + 1])
            y8F = y8.bitcast(F32)  # [128, V, 256] packed words
            tp = trp.tile([128, V, 2, 128], F32, tag="tp", name="tp")
            for v in range(V):
                for q in range(2):
                    nc.tensor.transpose(
                        tp[:, v, q, :], y8F[:, v, 128 * q : 128 * (q + 1)], identF)
            nc.scalar.copy(YT[s][:, :, :, 128 * c : 128 * (c + 1)], tp[:])

        def mm_block(acc_ap, s_rows, mc, s_cols, ncols, v, stop=True):
            for t in range(4):
                nc.tensor.matmul(
                    acc_ap,
                    YTr[s_rows][:, v, :, t, 128 * mc : 128 * (mc + 1)],
                    YTr[s_cols][:, v, :, t, 0:ncols],
                    start=(t == 0), stop=(stop and t == 3), perf_mode=PM.DoubleRow)

        def drain(acc, n, rslot, mc, contrib, cslot, col_ranges):
            nc.vector.reduce_max(
                rmcoll[:, :, rslot, mc, contrib], acc[:, :, :n],
                axis=mybir.AxisListType.X)
            if col_ranges == [(0, 0, GS)]:
                # full-width merge: both views in one strided op
                nc.vector.scalar_tensor_tensor(
                    cmx[:, :, cslot, :], acc[:], 1.0,
                    cmx[:, :, cslot, :], ALU.mult, ALU.max)
            else:
                for v in range(V):
                    for (alo, clo, w) in col_ranges:
                        nc.vector.scalar_tensor_tensor(
                            cmx[:, v, cslot, clo : clo + w],
                            acc[:, v, alo : alo + w], 1.0,
                            cmx[:, v, cslot, clo : clo + w],
                            ALU.mult, ALU.max)

        def off_chunk(s_rows, mc, s_cols, rslot, contrib, cslot, assist=False):
            acc = accp.tile([128, V, GS], F32, tag="acc", name="acc")
            for v in range(V):
                mm_block(acc[:, v, :], s_rows, mc, s_cols, GS, v)
            if assist:
                # ACT stages the PSUM block to fp16 SBUF (frees the PSUM bank
                # early, keeps PE dense); DVE reduces at 2-byte rate
                sc = scrp.tile([128, V, GS], F16, tag="scr", name="scr")
                nc.scalar.copy(sc[:], acc[:])
                nc.vector.reduce_max(
                    rmcoll[:, :, rslot, mc, contrib], sc[:],
                    axis=mybir.AxisListType.X)
                nc.vector.scalar_tensor_tensor(
                    cmx[:, :, cslot, :], sc[:], 1.0,
                    cmx[:, :, cslot, :], ALU.mult, ALU.max)
            else:
                drain(acc, GS, rslot, mc, contrib, cslot, [(0, 0, GS)])

        def diag_chunk(a):
            n = (a + 1) * 128
            acc = accp.tile([128, V, GS], F32, tag="acc", name="acc")
            for v in range(V):
                mm_block(acc[:, v, :n], S_R0, a, S_R0, n, v, stop=False)
                nc.tensor.matmul(
                    acc[:, v, 128 * a : 128 * (a + 1)], negI, identB,
                    start=False, stop=True)
            drain(acc, n, 0, a, 1, 2,
                  [(128 * b, 128 * b, 128) for b in range(a)])

        def col_allreduce(cslot):
            for v in range(V):
                nc.gpsimd.partition_all_reduce(
                    cmr[:, v, cslot, :], cmx[:, v, cslot, :],
                    channels=128, reduce_op=bass_isa.ReduceOp.max)

        def gram_for(s, c):
            if s == S_R0:
                diag_chunk(c)
                off_chunk(S_R0, c, S_C0, 0, 0, 0)     # u01
            elif s == S_C1:
                off_chunk(S_C1, c, S_R0, 2, 0, 2)     # u20
                if c == NCH - 1:
                    col_allreduce(2)                  # r0 cols (diag + u20)
            elif s == S_R1:
                # u31 streams with the arriving r1 chunks; after its last
                # merge, c0's allreduce overlaps the whole u32 replay, so
                # only c1's allreduce trails the final matmuls
                off_chunk(S_R1, c, S_C0, 1, 0, 0, assist=True)      # u31
                if c == NCH - 1:
                    col_allreduce(0)                   # c0 cols (u01 + u31)
                    for m in range(NCH):
                        off_chunk(S_R1, m, S_C1, 1, 1, 1, assist=True)  # u32
                    col_allreduce(1)                   # c1 cols (u32)

        # ---- skewed pipeline over all 16 chunks ----
        chunks = [(s, c) for s in range(4) for c in range(NCH)]
        for i, (s, c) in enumerate(chunks):
            stage_a(s, c, both_dve=(i % 2 == 1))
            if i > 0:
                ps, pc = chunks[i - 1]
                stage_b(ps, pc)
                gram_for(ps, pc)
        stage_b(*chunks[-1])
        gram_for(*chunks[-1])

        rmfin = const.tile([128, V, 3, NCH], F32, name="rmfin")
        nc.vector.reduce_max(rmfin[:], rmcoll[:], axis=mybir.AxisListType.X)
        nc.sync.dma_start(rm_d, rmfin.rearrange("p v r c -> p (v r c)"))
        nc.sync.dma_start(cm_d, cmr[0:1].rearrange("p v i j -> p (v i j)"))

    nc.compile()
    return nc


_CACHED = {}


def _consts():
    cstF = np.zeros((128, 129), np.float32)
    cstF[:, 0:128] = np.eye(128)
    cstF[:, 128] = EPS / (YSC * YSC)
    cstB = np.zeros((128, 256), np.float32)
    cstB[:, 0:128] = np.eye(128)
    cstB[:, 128:256] = MASKV * np.eye(128)
    import ml_dtypes
    return cstF, cstB.astype(ml_dtypes.bfloat16)


def _run(x, trace=False):
    x = np.ascontiguousarray(np.asarray(x, dtype=np.float32))
    assert x.shape == (B, V, D), x.shape
    if "nc" not in _CACHED:
        _CACHED["nc"] = build()
    nc = _CACHED["nc"]
    cstF, cstB = _consts()
    xr = x.reshape(G, GS, V, D)
    in_maps = []
    for r0, r1, c0, c1 in CORES:
        in_maps.append({
            "xg": np.ascontiguousarray(xr[[c0, r0, c1, r1]]),
            "cstF": cstF,
            "cstB": cstB,
        })
    res = bass_utils.run_bass_kernel_spmd(
        nc, in_maps, core_ids=list(range(NCORES)), trace=trace)

    M = np.full((V, B), MASKV, np.float32)
    for k, (r0, r1, c0, c1) in enumerate(CORES):
        rm = np.asarray(res.results[k]["rm"]).reshape(128, V, 3, NCH)
        cm = np.asarray(res.results[k]["cm"]).reshape(V, 3, GS)
        for v in range(V):
            for ri, g in ((0, r0), (1, r1), (2, c1)):
                seg = M[v, g * GS : (g + 1) * GS]
                np.maximum(seg, rm[:, v, ri, :].T.reshape(GS), out=seg)
            for ci, g in ((0, c0), (1, c1), (2, r0)):
                seg = M[v, g * GS : (g + 1) * GS]
                np.maximum(seg, cm[v, ci], out=seg)

    m = M.astype(np.float64) / (YSC * YSC)
    dist = np.sqrt(np.maximum(2.0 - 2.0 * m, 0.0))
    total = np.float32(-np.sum(np.mean(np.log(dist + EPS), axis=1)))
    return total, res


def kernel(student_global_cls_tokens):
    total, _ = _run(student_global_cls_tokens, trace=False)
    return np.asarray(total, dtype=np.float32)


# revision 24
# speedup vs baseline: 1.1951x; 1.1951x over previous
"""KoLeo-loss kernel for Trainium2, 8 NeuronCores — symmetric Gram + fp8.

Math: rows are L2-normalized; for unit vectors dist(a,b) = sqrt(2-2*a.b), so
the per-row NN distance needs only the row-max of the diagonal-masked cosine
Gram matrix.  G = Y Y^T is symmetric: each off-diagonal 512x512 group-block is
computed ONCE and yields row-max partials (free-dim reduce on DVE) plus
col-max partials (partition-dim reduce via GPSIMD partition_all_reduce).
This halves matmul FLOPs and HBM traffic vs computing full Gram rows.

Precision: Y is quantized to fp8e4 (x16 scale; dots then x256) and the Gram
runs as DoubleRow fp8 matmuls — 4 matmuls of K=256 per [128,512] block, 2x
the bf16 rate.  fp8 rounding moves the loss ~1e-3 relative (threshold 2e-2).
Layout: fp8 y is transposed as packed f32 words (4 fp8/word) giving
YT[p, v, q, row-word]; word-block q in {0,1} spans d = 4*(128q+p)+t.
DoubleRow k-subtile pairs run over q (plane stride - ISA-encodable, verified
on HW); the 4 K-phases run over byte index t.  Only 4 PE transposes/chunk.

Work split (8 groups of 512 rows): 28 off-diag group pairs + 8 diag
triangles.  Core k gets groups (r0, r1, c0, c1): units u01=(r0 x c0),
u31=(r1 x c0), u32=(r1 x c1), u20=(c1 x r0) [that pair computed transposed
so it row-streams off c1's chunks], plus the masked diagonal triangle of r0.
All 28 pairs + 8 diagonals covered (4 pairs twice — harmless for max).
Every core runs the IDENTICAL program; the host packs xg in processing
order [c0, r0, c1, r1] and maps the partial maxes back.

Pipelining: the front-end is emitted in two stages with a one-chunk skew
(A: dma + squares + rsqrt chain, B: fp8 stores + transposes + copyback +
dependent Gram blocks) so each in-order engine queue never head-of-line
blocks on another engine's freshly-issued work.  Engine balance per chunk:
ACT = square(v0) + sqrt + both fp8 scale-stores; DVE = square(v1) via
scalar_tensor_tensor-accum + reciprocal + copyback + all PSUM drains
(row reduce_max + col max-merge); GPSIMD = partition_all_reduce only.
"""

import os
import sys
from contextlib import ExitStack

import numpy as np

sys.path.insert(0, "/opt/trn_rl_repo")

import concourse.bass as bass
import concourse.mybir as mybir
import concourse.tile as tile
from concourse import bacc, bass_isa, bass_utils

F32 = mybir.dt.float32
F8 = mybir.dt.float8e4
BF16 = mybir.dt.bfloat16
AF = mybir.ActivationFunctionType
ALU = mybir.AluOpType
PM = mybir.MatmulPerfMode

B, V, D = 4096, 2, 1024
NCORES = 8
G, GS, NCH = 8, 512, 4
EPS = 1e-8
YSC = 16.0          # fp8 quantization scale for y; dots scale by YSC^2
MASKV = -1024.0     # below -YSC^2, dominated by any real dot

# per-core groups (r0, r1, c0, c1); r0 also carries the diagonal triangle.
CORES = [
    (0, 1, 2, 3),
    (1, 0, 4, 5),
    (6, 7, 0, 1),
    (2, 3, 4, 5),
    (3, 2, 6, 7),
    (4, 5, 6, 7),
    (5, 0, 1, 4),
    (7, 2, 3, 6),
]
# xg slot packing (= load/processing order): [c0, r0, c1, r1]
S_C0, S_R0, S_C1, S_R1 = 0, 1, 2, 3


def build():
    nc = bacc.Bacc("TRN2", debug=False)
    xg_d = nc.dram_tensor("xg", [4, GS, V, D], F32, kind="ExternalInput").ap()
    cstF_d = nc.dram_tensor("cstF", [128, 129], F32, kind="ExternalInput").ap()
    cstB_d = nc.dram_tensor("cstB", [128, 256], BF16, kind="ExternalInput").ap()
    rm_d = nc.dram_tensor("rm", [128, V * 3 * NCH], F32, kind="ExternalOutput").ap()
    cm_d = nc.dram_tensor("cm", [1, V * 3 * GS], F32, kind="ExternalOutput").ap()

    with ExitStack() as ctx:
        tc = ctx.enter_context(tile.TileContext(nc))
        const = ctx.enter_context(tc.tile_pool(name="const", bufs=1))
        xpool = ctx.enter_context(tc.tile_pool(name="xpool", bufs=8))
        ypool = ctx.enter_context(tc.tile_pool(name="ypool", bufs=4))
        sqpool = ctx.enter_context(tc.tile_pool(name="sqpool", bufs=3))
        sspool = ctx.enter_context(tc.tile_pool(name="sspool", bufs=4))
        accp = ctx.enter_context(tc.tile_pool(name="accp", bufs=3, space="PSUM"))
        trp = ctx.enter_context(tc.tile_pool(name="trp", bufs=2, space="PSUM"))

        cstF = const.tile([128, 129], F32, name="cstF")
        nc.sync.dma_start(cstF[:], cstF_d)
        identF = cstF[:, 0:128]
        epsb = cstF[:, 128:129]
        cstB = const.tile([128, 256], BF16, name="cstB")
        nc.sync.dma_start(cstB[:], cstB_d)
        identB = cstB[:, 0:128]
        negI = cstB[:, 128:256]

        # persistent transposed fp8 (word-packed) buffers, one per slot
        YT = [const.tile([128, V, 2, GS], F32, name=f"YT{s}") for s in range(4)]
        YTr = [t.bitcast(F8).rearrange("p v q (j t) -> p v q t j", t=4) for t in YT]

        # rmcoll[p, v, rslot, chunk, contrib]; rslots: 0=r0, 1=r1, 2=c1
        rmcoll = const.tile([128, V, 3, NCH, 2], F32, name="rmcoll")
        nc.gpsimd.memset(rmcoll[:], MASKV)
        # cmx[p, v, cslot, :]; cslots: 0=c0, 1=c1, 2=r0 (diag + u20)
        cmx = const.tile([128, V, 3, GS], F32, name="cmx")
        nc.gpsimd.memset(cmx[:], MASKV)
        cmr = const.tile([128, V, 3, GS], F32, name="cmr")

        state = {}

        def stage_a(s, c, both_dve):
            """DMA + norms for chunk (s, c).  Squares alternate between
            ACT+DVE and DVE-only chunks to balance the two engines."""
            xt = xpool.tile([128, V, D], F32, tag="xraw", name="xraw")
            nc.sync.dma_start(xt[:], xg_d[s, 128 * c : 128 * (c + 1)])
            ss = sspool.tile([128, V], F32, tag="ss", name="ss")
            if both_dve:
                sq0 = sqpool.tile([128, D], BF16, tag="sq0", name="sq0")
                nc.vector.scalar_tensor_tensor(
                    sq0[:], xt[:, 0, :], 1.0, xt[:, 0, :], ALU.mult, ALU.mult,
                    accum_out=ss[:, 0:1])
            else:
                sq0 = sqpool.tile([128, D], BF16, tag="sq0", name="sq0")
                nc.scalar.activation(
                    sq0[:], xt[:, 0, :], AF.Square, accum_out=ss[:, 0:1])
            sq1 = sqpool.tile([128, D], BF16, tag="sq1", name="sq1")
            nc.vector.scalar_tensor_tensor(
                sq1[:], xt[:, 1, :], 1.0, xt[:, 1, :], ALU.mult, ALU.mult,
                accum_out=ss[:, 1:2])
            # nrm = sqrt(ss+eps)/YSC so that rinv = YSC/|x|
            nrm = sspool.tile([128, V], F32, tag="nrm", name="nrm")
            nc.scalar.activation(
                nrm[:], ss[:], AF.Sqrt, bias=epsb, scale=1.0 / (YSC * YSC))
            rinv = sspool.tile([128, V], F32, tag="rinv", name="rinv")
            nc.vector.reciprocal(rinv[:], nrm[:])
            state[(s, c)] = (xt, rinv)

        def stage_b(s, c, split=False):
            """fp8 stores + transposes + copyback for chunk (s, c).
            split=True runs the v1 store on DVE in parallel with ACT's v0
            store (shorter serial chain; used for the first chunks)."""
            xt, rinv = state.pop((s, c))
            y8 = ypool.tile([128, V, D], F8, tag="y8", name="y8")
            nc.scalar.activation(
                y8[:, 0, :], xt[:, 0, :], AF.Copy, bias=0.0,
                scale=rinv[:, 0:1])
            if split:
                ybt = ypool.tile([128, D], BF16, tag="ybt", name="ybt")
                nc.vector.tensor_scalar_mul(ybt[:], xt[:, 1, :], rinv[:, 1:2])
                nc.vector.tensor_copy(y8[:, 1, :], ybt[:])
            else:
                nc.scalar.activation(
                    y8[:, 1, :], xt[:, 1, :], AF.Copy, bias=0.0,
                    scale=rinv[:, 1:2])
            y8F = y8.bitcast(F32)  # [128, V, 256] packed words
            tp = trp.tile([128, V, 2, 128], F32, tag="tp", name="tp")
            for v in range(V):
                for q in range(2):
                    nc.tensor.transpose(
                        tp[:, v, q, :], y8F[:, v, 128 * q : 128 * (q + 1)], identF)
            nc.scalar.copy(YT[s][:, :, :, 128 * c : 128 * (c + 1)], tp[:])

        def mm_block(acc_ap, s_rows, mc, s_cols, ncols, v, stop=True):
            for t in range(4):
                nc.tensor.matmul(
                    acc_ap,
                    YTr[s_rows][:, v, :, t, 128 * mc : 128 * (mc + 1)],
                    YTr[s_cols][:, v, :, t, 0:ncols],
                    start=(t == 0), stop=(stop and t == 3), perf_mode=PM.DoubleRow)

        def drain(acc, n, rslot, mc, contrib, cslot, col_ranges):
            nc.vector.reduce_max(
                rmcoll[:, :, rslot, mc, contrib], acc[:, :, :n],
                axis=mybir.AxisListType.X)
            if col_ranges == [(0, 0, GS)]:
                # full-width merge: both views in one strided op
                nc.vector.scalar_tensor_tensor(
                    cmx[:, :, cslot, :], acc[:], 1.0,
                    cmx[:, :, cslot, :], ALU.mult, ALU.max)
            else:
                for v in range(V):
                    for (alo, clo, w) in col_ranges:
                        nc.vector.scalar_tensor_tensor(
                            cmx[:, v, cslot, clo : clo + w],
                            acc[:, v, alo : alo + w], 1.0,
                            cmx[:, v, cslot, clo : clo + w],
                            ALU.mult, ALU.max)

        def off_chunk(s_rows, mc, s_cols, rslot, contrib, cslot):
            acc = accp.tile([128, V, GS], F32, tag="acc", name="acc")
            for v in range(V):
                mm_block(acc[:, v, :], s_rows, mc, s_cols, GS, v)
            drain(acc, GS, rslot, mc, contrib, cslot, [(0, 0, GS)])

        def diag_chunk(a):
            n = (a + 1) * 128
            acc = accp.tile([128, V, GS], F32, tag="acc", name="acc")
            for v in range(V):
                mm_block(acc[:, v, :n], S_R0, a, S_R0, n, v, stop=False)
                nc.tensor.matmul(
                    acc[:, v, 128 * a : 128 * (a + 1)], negI, identB,
                    start=False, stop=True)
            drain(acc, n, 0, a, 1, 2,
                  [(128 * b, 128 * b, 128) for b in range(a)])

        def col_allreduce(cslot):
            for v in range(V):
                nc.gpsimd.partition_all_reduce(
                    cmr[:, v, cslot, :], cmx[:, v, cslot, :],
                    channels=128, reduce_op=bass_isa.ReduceOp.max)

        def gram_for(s, c):
            if s == S_R0:
                diag_chunk(c)
                off_chunk(S_R0, c, S_C0, 0, 0, 0)     # u01
            elif s == S_C1:
                off_chunk(S_C1, c, S_R0, 2, 0, 2)     # u20
                if c == NCH - 1:
                    col_allreduce(2)                  # r0 cols (diag + u20)
            elif s == S_R1:
                if c < NCH - 1:
                    off_chunk(S_R1, c, S_C0, 1, 0, 0)  # u31
                    off_chunk(S_R1, c, S_C1, 1, 1, 1)  # u32
                else:
                    # last chunk: c0's allreduce fires after u31's drain and
                    # overlaps u32's matmuls+drain; only c1's allreduce trails
                    off_chunk(S_R1, c, S_C0, 1, 0, 0)
                    col_allreduce(0)                   # c0 cols (u01 + u31)
                    off_chunk(S_R1, c, S_C1, 1, 1, 1)
                    col_allreduce(1)                   # c1 cols (u32)

        # ---- skewed pipeline over all 16 chunks ----
        chunks = [(s, c) for s in range(4) for c in range(NCH)]
        for i, (s, c) in enumerate(chunks):
            stage_a(s, c, both_dve=(i % 2 == 1))
            if i > 0:
                ps, pc = chunks[i - 1]
                stage_b(ps, pc, split=(i <= 2))
                gram_for(ps, pc)
        stage_b(*chunks[-1])
        gram_for(*chunks[-1])

        rmfin = const.tile([128, V, 3, NCH], F32, name="rmfin")
        nc.vector.reduce_max(rmfin[:], rmcoll[:], axis=mybir.AxisListType.X)
        nc.sync.dma_start(rm_d, rmfin.rearrange("p v r c -> p (v r c)"))
        nc.sync.dma_start(cm_d, cmr[0:1].rearrange("p v i j -> p (v i j)"))

    nc.compile()
    return nc


_CACHED = {}


def _consts():
    cstF = np.zeros((128, 129), np.float32)
    cstF[:, 0:128] = np.eye(128)
    cstF[:, 128] = EPS / (YSC * YSC)
    cstB = np.zeros((128, 256), np.float32)
    cstB[:, 0:128] = np.eye(128)
    cstB[:, 128:256] = MASKV * np.eye(128)
    import ml_dtypes
    return cstF, cstB.astype(ml_dtypes.bfloat16)


def _run(x, trace=False):
    x = np.ascontiguousarray(np.asarray(x, dtype=np.float32))
    assert x.shape == (B, V, D), x.shape
    if "nc" not in _CACHED:
        _CACHED["nc"] = build()
    nc = _CACHED["nc"]
    cstF, cstB = _consts()
    xr = x.reshape(G, GS, V, D)
    in_maps = []
    for r0, r1, c0, c1 in CORES:
        in_maps.append({
            "xg": np.ascontiguousarray(xr[[c0, r0, c1, r1]]),
            "cstF": cstF,
            "cstB": cstB,
        })
    res = bass_utils.run_bass_kernel_spmd(
        nc, in_maps, core_ids=list(range(NCORES)), trace=trace)

    M = np.full((V, B), MASKV, np.float32)
    for k, (r0, r1, c0, c1) in enumerate(CORES):
        rm = np.asarray(res.results[k]["rm"]).reshape(128, V, 3, NCH)
        cm = np.asarray(res.results[k]["cm"]).reshape(V, 3, GS)
        for v in range(V):
            for ri, g in ((0, r0), (1, r1), (2, c1)):
                seg = M[v, g * GS : (g + 1) * GS]
                np.maximum(seg, rm[:, v, ri, :].T.reshape(GS), out=seg)
            for ci, g in ((0, c0), (1, c1), (2, r0)):
                seg = M[v, g * GS : (g + 1) * GS]
                np.maximum(seg, cm[v, ci], out=seg)

    m = M.astype(np.float64) / (YSC * YSC)
    dist = np.sqrt(np.maximum(2.0 - 2.0 * m, 0.0))
    total = np.float32(-np.sum(np.mean(np.log(dist + EPS), axis=1)))
    return total, res


def kernel(student_global_cls_tokens):
    total, _ = _run(student_global_cls_tokens, trace=False)
    return np.asarray(total, dtype=np.float32)


# revision 25
# speedup vs baseline: 1.2067x; 1.0097x over previous
"""KoLeo-loss kernel for Trainium2, 8 NeuronCores — symmetric Gram + fp8.

Math: rows are L2-normalized; for unit vectors dist(a,b) = sqrt(2-2*a.b), so
the per-row NN distance needs only the row-max of the diagonal-masked cosine
Gram matrix.  G = Y Y^T is symmetric: each off-diagonal 512x512 group-block is
computed ONCE and yields row-max partials (free-dim reduce on DVE) plus
col-max partials (partition-dim reduce via GPSIMD partition_all_reduce).
This halves matmul FLOPs and HBM traffic vs computing full Gram rows.

Precision: Y is quantized to fp8e4 (x16 scale; dots then x256) and the Gram
runs as DoubleRow fp8 matmuls — 4 matmuls of K=256 per [128,512] block, 2x
the bf16 rate.  fp8 rounding moves the loss ~1e-3 relative (threshold 2e-2).
Layout: fp8 y is transposed as packed f32 words (4 fp8/word) giving
YT[p, v, q, row-word]; word-block q in {0,1} spans d = 4*(128q+p)+t.
DoubleRow k-subtile pairs run over q (plane stride - ISA-encodable, verified
on HW); the 4 K-phases run over byte index t.  Only 4 PE transposes/chunk.

Work split (8 groups of 512 rows): 28 off-diag group pairs + 8 diag
triangles.  Core k gets groups (r0, r1, c0, c1): units u01=(r0 x c0),
u31=(r1 x c0), u32=(r1 x c1), u20=(c1 x r0) [that pair computed transposed
so it row-streams off c1's chunks], plus the masked diagonal triangle of r0.
All 28 pairs + 8 diagonals covered (4 pairs twice — harmless for max).
Every core runs the IDENTICAL program; the host packs xg in processing
order [c0, r0, c1, r1] and maps the partial maxes back.

Pipelining: the front-end is emitted in two stages with a one-chunk skew
(A: dma + squares + rsqrt chain, B: fp8 stores + transposes + copyback +
dependent Gram blocks) so each in-order engine queue never head-of-line
blocks on another engine's freshly-issued work.  Engine balance per chunk:
ACT = square(v0) + sqrt + both fp8 scale-stores; DVE = square(v1) via
scalar_tensor_tensor-accum + reciprocal + copyback + all PSUM drains
(row reduce_max + col max-merge); GPSIMD = partition_all_reduce only.
"""

import os
import sys
from contextlib import ExitStack

import numpy as np

sys.path.insert(0, "/opt/trn_rl_repo")

import concourse.bass as bass
import concourse.mybir as mybir
import concourse.tile as tile
from concourse import bacc, bass_isa, bass_utils

F32 = mybir.dt.float32
F8 = mybir.dt.float8e4
BF16 = mybir.dt.bfloat16
AF = mybir.ActivationFunctionType
ALU = mybir.AluOpType
PM = mybir.MatmulPerfMode

B, V, D = 4096, 2, 1024
NCORES = 8
G, GS, NCH = 8, 512, 4
EPS = 1e-8
YSC = 16.0          # fp8 quantization scale for y; dots scale by YSC^2
MASKV = -1024.0     # below -YSC^2, dominated by any real dot

# per-core groups (r0, r1, c0, c1); r0 also carries the diagonal triangle.
CORES = [
    (0, 1, 2, 3),
    (1, 0, 4, 5),
    (6, 7, 0, 1),
    (2, 3, 4, 5),
    (3, 2, 6, 7),
    (4, 5, 6, 7),
    (5, 0, 1, 4),
    (7, 2, 3, 6),
]
# xg slot packing (= load/processing order): [c0, r0, c1, r1]
S_C0, S_R0, S_C1, S_R1 = 0, 1, 2, 3


def build():
    nc = bacc.Bacc("TRN2", debug=False)
    xg_d = nc.dram_tensor("xg", [4, GS, V, D], F32, kind="ExternalInput").ap()
    cstF_d = nc.dram_tensor("cstF", [128, 129], F32, kind="ExternalInput").ap()
    cstB_d = nc.dram_tensor("cstB", [128, 256], BF16, kind="ExternalInput").ap()
    rm_d = nc.dram_tensor("rm", [128, V * 3 * NCH], F32, kind="ExternalOutput").ap()
    cm_d = nc.dram_tensor("cm", [1, V * 3 * GS], F32, kind="ExternalOutput").ap()

    with ExitStack() as ctx:
        tc = ctx.enter_context(tile.TileContext(nc))
        const = ctx.enter_context(tc.tile_pool(name="const", bufs=1))
        xpool = ctx.enter_context(tc.tile_pool(name="xpool", bufs=8))
        ypool = ctx.enter_context(tc.tile_pool(name="ypool", bufs=6))
        sqpool = ctx.enter_context(tc.tile_pool(name="sqpool", bufs=4))
        sspool = ctx.enter_context(tc.tile_pool(name="sspool", bufs=6))
        accp = ctx.enter_context(tc.tile_pool(name="accp", bufs=3, space="PSUM"))
        trp = ctx.enter_context(tc.tile_pool(name="trp", bufs=2, space="PSUM"))

        cstF = const.tile([128, 129], F32, name="cstF")
        nc.sync.dma_start(cstF[:], cstF_d)
        identF = cstF[:, 0:128]
        epsb = cstF[:, 128:129]
        cstB = const.tile([128, 256], BF16, name="cstB")
        nc.sync.dma_start(cstB[:], cstB_d)
        identB = cstB[:, 0:128]
        negI = cstB[:, 128:256]

        # persistent transposed fp8 (word-packed) buffers, one per slot
        YT = [const.tile([128, V, 2, GS], F32, name=f"YT{s}") for s in range(4)]
        YTr = [t.bitcast(F8).rearrange("p v q (j t) -> p v q t j", t=4) for t in YT]

        # rmcoll[p, v, rslot, chunk, contrib]; rslots: 0=r0, 1=r1, 2=c1
        rmcoll = const.tile([128, V, 3, NCH, 2], F32, name="rmcoll")
        nc.gpsimd.memset(rmcoll[:], MASKV)
        # cmx[p, v, cslot, :]; cslots: 0=c0, 1=c1, 2=r0 (diag + u20)
        cmx = const.tile([128, V, 3, GS], F32, name="cmx")
        nc.gpsimd.memset(cmx[:], MASKV)
        cmr = const.tile([128, V, 3, GS], F32, name="cmr")

        state = {}

        def stage_a(s, c, both_dve):
            """DMA + norms for chunk (s, c).  Squares alternate between
            ACT+DVE and DVE-only chunks to balance the two engines."""
            xt = xpool.tile([128, V, D], F32, tag="xraw", name="xraw")
            nc.sync.dma_start(xt[:], xg_d[s, 128 * c : 128 * (c + 1)])
            ss = sspool.tile([128, V], F32, tag="ss", name="ss")
            if both_dve:
                sq0 = sqpool.tile([128, D], BF16, tag="sq0", name="sq0")
                nc.vector.scalar_tensor_tensor(
                    sq0[:], xt[:, 0, :], 1.0, xt[:, 0, :], ALU.mult, ALU.mult,
                    accum_out=ss[:, 0:1])
            else:
                sq0 = sqpool.tile([128, D], BF16, tag="sq0", name="sq0")
                nc.scalar.activation(
                    sq0[:], xt[:, 0, :], AF.Square, accum_out=ss[:, 0:1])
            sq1 = sqpool.tile([128, D], BF16, tag="sq1", name="sq1")
            nc.vector.scalar_tensor_tensor(
                sq1[:], xt[:, 1, :], 1.0, xt[:, 1, :], ALU.mult, ALU.mult,
                accum_out=ss[:, 1:2])
            # nrm = sqrt(ss+eps)/YSC so that rinv = YSC/|x|
            nrm = sspool.tile([128, V], F32, tag="nrm", name="nrm")
            nc.scalar.activation(
                nrm[:], ss[:], AF.Sqrt, bias=epsb, scale=1.0 / (YSC * YSC))
            rinv = sspool.tile([128, V], F32, tag="rinv", name="rinv")
            nc.vector.reciprocal(rinv[:], nrm[:])
            state[(s, c)] = (xt, rinv)

        def stage_b(s, c, split=False):
            """fp8 stores + transposes + copyback for chunk (s, c).
            split=True runs the v1 store on DVE in parallel with ACT's v0
            store (shorter serial chain; used for the first chunks)."""
            xt, rinv = state.pop((s, c))
            y8 = ypool.tile([128, V, D], F8, tag="y8", name="y8")
            nc.scalar.activation(
                y8[:, 0, :], xt[:, 0, :], AF.Copy, bias=0.0,
                scale=rinv[:, 0:1])
            if split:
                ybt = ypool.tile([128, D], BF16, tag="ybt", name="ybt")
                nc.vector.tensor_scalar_mul(ybt[:], xt[:, 1, :], rinv[:, 1:2])
                nc.vector.tensor_copy(y8[:, 1, :], ybt[:])
            else:
                nc.scalar.activation(
                    y8[:, 1, :], xt[:, 1, :], AF.Copy, bias=0.0,
                    scale=rinv[:, 1:2])
            y8F = y8.bitcast(F32)  # [128, V, 256] packed words
            tp = trp.tile([128, V, 2, 128], F32, tag="tp", name="tp")
            for v in range(V):
                for q in range(2):
                    nc.tensor.transpose(
                        tp[:, v, q, :], y8F[:, v, 128 * q : 128 * (q + 1)], identF)
            nc.scalar.copy(YT[s][:, :, :, 128 * c : 128 * (c + 1)], tp[:])

        def mm_block(acc_ap, s_rows, mc, s_cols, ncols, v, stop=True):
            for t in range(4):
                nc.tensor.matmul(
                    acc_ap,
                    YTr[s_rows][:, v, :, t, 128 * mc : 128 * (mc + 1)],
                    YTr[s_cols][:, v, :, t, 0:ncols],
                    start=(t == 0), stop=(stop and t == 3), perf_mode=PM.DoubleRow)

        def drain(acc, n, rslot, mc, contrib, cslot, col_ranges):
            nc.vector.reduce_max(
                rmcoll[:, :, rslot, mc, contrib], acc[:, :, :n],
                axis=mybir.AxisListType.X)
            if col_ranges == [(0, 0, GS)]:
                # full-width merge: both views in one strided op
                nc.vector.scalar_tensor_tensor(
                    cmx[:, :, cslot, :], acc[:], 1.0,
                    cmx[:, :, cslot, :], ALU.mult, ALU.max)
            else:
                for v in range(V):
                    for (alo, clo, w) in col_ranges:
                        nc.vector.scalar_tensor_tensor(
                            cmx[:, v, cslot, clo : clo + w],
                            acc[:, v, alo : alo + w], 1.0,
                            cmx[:, v, cslot, clo : clo + w],
                            ALU.mult, ALU.max)

        def off_chunk(s_rows, mc, s_cols, rslot, contrib, cslot):
            acc = accp.tile([128, V, GS], F32, tag="acc", name="acc")
            for v in range(V):
                mm_block(acc[:, v, :], s_rows, mc, s_cols, GS, v)
            drain(acc, GS, rslot, mc, contrib, cslot, [(0, 0, GS)])

        def diag_chunk(a):
            n = (a + 1) * 128
            acc = accp.tile([128, V, GS], F32, tag="acc", name="acc")
            for v in range(V):
                mm_block(acc[:, v, :n], S_R0, a, S_R0, n, v, stop=False)
                nc.tensor.matmul(
                    acc[:, v, 128 * a : 128 * (a + 1)], negI, identB,
                    start=False, stop=True)
            drain(acc, n, 0, a, 1, 2,
                  [(0, 0, 128 * a)] if a > 0 else [])

        def col_allreduce(cslot):
            for v in range(V):
                nc.gpsimd.partition_all_reduce(
                    cmr[:, v, cslot, :], cmx[:, v, cslot, :],
                    channels=128, reduce_op=bass_isa.ReduceOp.max)

        def gram_for(s, c):
            if s == S_R0:
                diag_chunk(c)
                off_chunk(S_R0, c, S_C0, 0, 0, 0)     # u01
            elif s == S_C1:
                off_chunk(S_C1, c, S_R0, 2, 0, 2)     # u20
                if c == NCH - 1:
                    col_allreduce(2)                  # r0 cols (diag + u20)
            elif s == S_R1:
                if c < NCH - 1:
                    off_chunk(S_R1, c, S_C0, 1, 0, 0)  # u31
                    off_chunk(S_R1, c, S_C1, 1, 1, 1)  # u32
                else:
                    # last chunk: c0's allreduce fires after u31's drain and
                    # overlaps u32's matmuls+drain; only c1's allreduce trails
                    off_chunk(S_R1, c, S_C0, 1, 0, 0)
                    col_allreduce(0)                   # c0 cols (u01 + u31)
                    off_chunk(S_R1, c, S_C1, 1, 1, 1)
                    col_allreduce(1)                   # c1 cols (u32)

        # ---- skewed pipeline over all 16 chunks ----
        chunks = [(s, c) for s in range(4) for c in range(NCH)]
        for i, (s, c) in enumerate(chunks):
            stage_a(s, c, both_dve=(i % 2 == 1))
            if i > 0:
                ps, pc = chunks[i - 1]
                stage_b(ps, pc, split=(i <= 2))
                gram_for(ps, pc)
        stage_b(*chunks[-1])
        gram_for(*chunks[-1])

        rmfin = const.tile([128, V, 3, NCH], F32, name="rmfin")
        nc.vector.reduce_max(rmfin[:], rmcoll[:], axis=mybir.AxisListType.X)
        nc.sync.dma_start(rm_d, rmfin.rearrange("p v r c -> p (v r c)"))
        nc.sync.dma_start(cm_d, cmr[0:1].rearrange("p v i j -> p (v i j)"))

    nc.compile()
    return nc


_CACHED = {}


def _consts():
    cstF = np.zeros((128, 129), np.float32)
    cstF[:, 0:128] = np.eye(128)
    cstF[:, 128] = EPS / (YSC * YSC)
    cstB = np.zeros((128, 256), np.float32)
    cstB[:, 0:128] = np.eye(128)
    cstB[:, 128:256] = MASKV * np.eye(128)
    import ml_dtypes
    return cstF, cstB.astype(ml_dtypes.bfloat16)


def _run(x, trace=False):
    x = np.ascontiguousarray(np.asarray(x, dtype=np.float32))
    assert x.shape == (B, V, D), x.shape
    if "nc" not in _CACHED:
        _CACHED["nc"] = build()
    nc = _CACHED["nc"]
    cstF, cstB = _consts()
    xr = x.reshape(G, GS, V, D)
    in_maps = []
    for r0, r1, c0, c1 in CORES:
        in_maps.append({
            "xg": np.ascontiguousarray(xr[[c0, r0, c1, r1]]),
            "cstF": cstF,
            "cstB": cstB,
        })
    res = bass_utils.run_bass_kernel_spmd(
        nc, in_maps, core_ids=list(range(NCORES)), trace=trace)

    M = np.full((V, B), MASKV, np.float32)
    for k, (r0, r1, c0, c1) in enumerate(CORES):
        rm = np.asarray(res.results[k]["rm"]).reshape(128, V, 3, NCH)
        cm = np.asarray(res.results[k]["cm"]).reshape(V, 3, GS)
        for v in range(V):
            for ri, g in ((0, r0), (1, r1), (2, c1)):
                seg = M[v, g * GS : (g + 1) * GS]
                np.maximum(seg, rm[:, v, ri, :].T.reshape(GS), out=seg)
            for ci, g in ((0, c0), (1, c1), (2, r0)):
                seg = M[v, g * GS : (g + 1) * GS]
                np.maximum(seg, cm[v, ci], out=seg)

    m = M.astype(np.float64) / (YSC * YSC)
    dist = np.sqrt(np.maximum(2.0 - 2.0 * m, 0.0))
    total = np.float32(-np.sum(np.mean(np.log(dist + EPS), axis=1)))
    return total, res


def kernel(student_global_cls_tokens):
    total, _ = _run(student_global_cls_tokens, trace=False)
    return np.asarray(total, dtype=np.float32)


# revision 26
# speedup vs baseline: 1.2530x; 1.0384x over previous
"""KoLeo-loss kernel for Trainium2, 8 NeuronCores — symmetric Gram + fp8.

Math: rows are L2-normalized; for unit vectors dist(a,b) = sqrt(2-2*a.b), so
the per-row NN distance needs only the row-max of the diagonal-masked cosine
Gram matrix.  G = Y Y^T is symmetric: each off-diagonal 512x512 group-block is
computed ONCE and yields row-max partials (free-dim reduce on DVE) plus
col-max partials (partition-dim reduce via GPSIMD partition_all_reduce).
This halves matmul FLOPs and HBM traffic vs computing full Gram rows.

Precision: Y is quantized to fp8e4 (x16 scale; dots then x256) and the Gram
runs as DoubleRow fp8 matmuls — 4 matmuls of K=256 per [128,512] block, 2x
the bf16 rate.  fp8 rounding moves the loss ~1e-3 relative (threshold 2e-2).
Layout: fp8 y is transposed as packed f32 words (4 fp8/word) giving
YT[p, v, q, row-word]; word-block q in {0,1} spans d = 4*(128q+p)+t.
DoubleRow k-subtile pairs run over q (plane stride - ISA-encodable, verified
on HW); the 4 K-phases run over byte index t.  Only 4 PE transposes/chunk.

Work split (8 groups of 512 rows): 28 off-diag group pairs + 8 diag
triangles.  Core k gets groups (r0, r1, c0, c1): units u01=(r0 x c0),
u31=(r1 x c0), u32=(r1 x c1), u20=(c1 x r0) [that pair computed transposed
so it row-streams off c1's chunks], plus the masked diagonal triangle of r0.
All 28 pairs + 8 diagonals covered (4 pairs twice — harmless for max).
Every core runs the IDENTICAL program; the host packs xg in processing
order [c0, r0, c1, r1] and maps the partial maxes back.

Pipelining: the front-end is emitted in two stages with a one-chunk skew
(A: dma + squares + rsqrt chain, B: fp8 stores + transposes + copyback +
dependent Gram blocks) so each in-order engine queue never head-of-line
blocks on another engine's freshly-issued work.  Engine balance per chunk:
ACT = square(v0) + sqrt + both fp8 scale-stores; DVE = square(v1) via
scalar_tensor_tensor-accum + reciprocal + copyback + all PSUM drains
(row reduce_max + col max-merge); GPSIMD = partition_all_reduce only.
"""

import os
import sys
from contextlib import ExitStack

import numpy as np

sys.path.insert(0, "/opt/trn_rl_repo")

import concourse.bass as bass
import concourse.mybir as mybir
import concourse.tile as tile
from concourse import bacc, bass_isa, bass_utils

F32 = mybir.dt.float32
F8 = mybir.dt.float8e4
BF16 = mybir.dt.bfloat16
AF = mybir.ActivationFunctionType
ALU = mybir.AluOpType
PM = mybir.MatmulPerfMode

B, V, D = 4096, 2, 1024
NCORES = 8
G, GS, NCH = 8, 512, 4
EPS = 1e-8
YSC = 16.0          # fp8 quantization scale for y; dots scale by YSC^2
MASKV = -1024.0     # below -YSC^2, dominated by any real dot

# per-core groups (r0, r1, c0, c1); r0 also carries the diagonal triangle.
CORES = [
    (0, 1, 2, 3),
    (1, 0, 4, 5),
    (6, 7, 0, 1),
    (2, 3, 4, 5),
    (3, 2, 6, 7),
    (4, 5, 6, 7),
    (5, 0, 1, 4),
    (7, 2, 3, 6),
]
# xg slot packing (= load/processing order): [c0, r0, c1, r1]
S_C0, S_R0, S_C1, S_R1 = 0, 1, 2, 3


def build():
    nc = bacc.Bacc("TRN2", debug=False)
    xg_d = nc.dram_tensor("xg", [4, GS, V, D], F32, kind="ExternalInput").ap()
    cstF_d = nc.dram_tensor("cstF", [128, 129], F32, kind="ExternalInput").ap()
    cstB_d = nc.dram_tensor("cstB", [128, 256], BF16, kind="ExternalInput").ap()
    rm_d = nc.dram_tensor("rm", [128, V * 3 * NCH], F32, kind="ExternalOutput").ap()
    cm_d = nc.dram_tensor("cm", [1, V * 3 * GS], F32, kind="ExternalOutput").ap()
    cmt_d = nc.dram_tensor("cmt", [128, V * 4], F32, kind="ExternalOutput").ap()

    with ExitStack() as ctx:
        tc = ctx.enter_context(tile.TileContext(nc))
        const = ctx.enter_context(tc.tile_pool(name="const", bufs=1))
        xpool = ctx.enter_context(tc.tile_pool(name="xpool", bufs=8))
        ypool = ctx.enter_context(tc.tile_pool(name="ypool", bufs=6))
        sqpool = ctx.enter_context(tc.tile_pool(name="sqpool", bufs=4))
        sspool = ctx.enter_context(tc.tile_pool(name="sspool", bufs=6))
        accp = ctx.enter_context(tc.tile_pool(name="accp", bufs=3, space="PSUM"))
        trp = ctx.enter_context(tc.tile_pool(name="trp", bufs=2, space="PSUM"))

        cstF = const.tile([128, 129], F32, name="cstF")
        identF = cstF[:, 0:128]
        epsb = cstF[:, 128:129]
        cstB = const.tile([128, 256], BF16, name="cstB")
        identB = cstB[:, 0:128]
        negI = cstB[:, 128:256]

        def load_consts():
            nc.sync.dma_start(cstF[:], cstF_d)
            nc.sync.dma_start(cstB[:], cstB_d)

        # persistent transposed fp8 (word-packed) buffers, one per slot
        YT = [const.tile([128, V, 2, GS], F32, name=f"YT{s}") for s in range(4)]
        YTr = [t.bitcast(F8).rearrange("p v q (j t) -> p v q t j", t=4) for t in YT]

        # rmcoll[p, v, rslot, chunk, contrib]; rslots: 0=r0, 1=r1, 2=c1
        rmcoll = const.tile([128, V, 3, NCH, 2], F32, name="rmcoll")
        nc.gpsimd.memset(rmcoll[:], MASKV)
        # cmx[p, v, cslot, :]; cslots: 0=c0, 1=c1, 2=r0 (diag + u20)
        cmx = const.tile([128, V, 3, GS], F32, name="cmx")
        nc.gpsimd.memset(cmx[:], MASKV)
        cmr = const.tile([128, V, 3, GS], F32, name="cmr")
        cmt = const.tile([128, V, 4], F32, name="cmt")

        state = {}

        def stage_a(s, c, both_dve):
            """DMA + norms for chunk (s, c).  Squares alternate between
            ACT+DVE and DVE-only chunks to balance the two engines."""
            xt = xpool.tile([128, V, D], F32, tag="xraw", name="xraw")
            nc.sync.dma_start(xt[:], xg_d[s, 128 * c : 128 * (c + 1)])
            ss = sspool.tile([128, V], F32, tag="ss", name="ss")
            if both_dve:
                sq0 = sqpool.tile([128, D], BF16, tag="sq0", name="sq0")
                nc.vector.scalar_tensor_tensor(
                    sq0[:], xt[:, 0, :], 1.0, xt[:, 0, :], ALU.mult, ALU.mult,
                    accum_out=ss[:, 0:1])
            else:
                sq0 = sqpool.tile([128, D], BF16, tag="sq0", name="sq0")
                nc.scalar.activation(
                    sq0[:], xt[:, 0, :], AF.Square, accum_out=ss[:, 0:1])
            sq1 = sqpool.tile([128, D], BF16, tag="sq1", name="sq1")
            nc.vector.scalar_tensor_tensor(
                sq1[:], xt[:, 1, :], 1.0, xt[:, 1, :], ALU.mult, ALU.mult,
                accum_out=ss[:, 1:2])
            # nrm = sqrt(ss+eps)/YSC so that rinv = YSC/|x|
            nrm = sspool.tile([128, V], F32, tag="nrm", name="nrm")
            nc.scalar.activation(
                nrm[:], ss[:], AF.Sqrt, bias=epsb, scale=1.0 / (YSC * YSC))
            rinv = sspool.tile([128, V], F32, tag="rinv", name="rinv")
            nc.vector.reciprocal(rinv[:], nrm[:])
            state[(s, c)] = (xt, rinv)

        def stage_b(s, c, split=False):
            """fp8 stores + transposes + copyback for chunk (s, c).
            split=True runs the v1 store on DVE in parallel with ACT's v0
            store (shorter serial chain; used for the first chunks)."""
            xt, rinv = state.pop((s, c))
            y8 = ypool.tile([128, V, D], F8, tag="y8", name="y8")
            nc.scalar.activation(
                y8[:, 0, :], xt[:, 0, :], AF.Copy, bias=0.0,
                scale=rinv[:, 0:1])
            if split:
                ybt = ypool.tile([128, D], BF16, tag="ybt", name="ybt")
                nc.vector.tensor_scalar_mul(ybt[:], xt[:, 1, :], rinv[:, 1:2])
                nc.vector.tensor_copy(y8[:, 1, :], ybt[:])
            else:
                nc.scalar.activation(
                    y8[:, 1, :], xt[:, 1, :], AF.Copy, bias=0.0,
                    scale=rinv[:, 1:2])
            y8F = y8.bitcast(F32)  # [128, V, 256] packed words
            tp = trp.tile([128, V, 2, 128], F32, tag="tp", name="tp")
            for v in range(V):
                for q in range(2):
                    nc.tensor.transpose(
                        tp[:, v, q, :], y8F[:, v, 128 * q : 128 * (q + 1)], identF)
            nc.scalar.copy(YT[s][:, :, :, 128 * c : 128 * (c + 1)], tp[:])

        def mm_block(acc_ap, s_rows, mc, s_cols, ncols, v, stop=True):
            for t in range(4):
                nc.tensor.matmul(
                    acc_ap,
                    YTr[s_rows][:, v, :, t, 128 * mc : 128 * (mc + 1)],
                    YTr[s_cols][:, v, :, t, 0:ncols],
                    start=(t == 0), stop=(stop and t == 3), perf_mode=PM.DoubleRow)

        def drain(acc, n, rslot, mc, contrib, cslot, col_ranges):
            nc.vector.reduce_max(
                rmcoll[:, :, rslot, mc, contrib], acc[:, :, :n],
                axis=mybir.AxisListType.X)
            if col_ranges == [(0, 0, GS)]:
                # full-width merge: both views in one strided op
                nc.vector.scalar_tensor_tensor(
                    cmx[:, :, cslot, :], acc[:], 1.0,
                    cmx[:, :, cslot, :], ALU.mult, ALU.max)
            else:
                for v in range(V):
                    for (alo, clo, w) in col_ranges:
                        nc.vector.scalar_tensor_tensor(
                            cmx[:, v, cslot, clo : clo + w],
                            acc[:, v, alo : alo + w], 1.0,
                            cmx[:, v, cslot, clo : clo + w],
                            ALU.mult, ALU.max)

        def off_chunk(s_rows, mc, s_cols, rslot, contrib, cslot):
            acc = accp.tile([128, V, GS], F32, tag="acc", name="acc")
            for v in range(V):
                mm_block(acc[:, v, :], s_rows, mc, s_cols, GS, v)
            drain(acc, GS, rslot, mc, contrib, cslot, [(0, 0, GS)])

        def diag_chunk(a):
            n = (a + 1) * 128
            acc = accp.tile([128, V, GS], F32, tag="acc", name="acc")
            for v in range(V):
                mm_block(acc[:, v, :n], S_R0, a, S_R0, n, v, stop=False)
                nc.tensor.matmul(
                    acc[:, v, 128 * a : 128 * (a + 1)], negI, identB,
                    start=False, stop=True)
            drain(acc, n, 0, a, 1, 2,
                  [(0, 0, 128 * a)] if a > 0 else [])

        def col_allreduce(cslot):
            for v in range(V):
                nc.gpsimd.partition_all_reduce(
                    cmr[:, v, cslot, :], cmx[:, v, cslot, :],
                    channels=128, reduce_op=bass_isa.ReduceOp.max)

        def gram_for(s, c):
            if s == S_R0:
                diag_chunk(c)
                off_chunk(S_R0, c, S_C0, 0, 0, 0)     # u01
            elif s == S_C1:
                off_chunk(S_C1, c, S_R0, 2, 0, 2)     # u20
                if c == NCH - 1:
                    col_allreduce(2)                  # r0 cols (diag + u20)
            elif s == S_R1:
                if c < NCH - 1:
                    off_chunk(S_R1, c, S_C0, 1, 0, 0)  # u31
                    off_chunk(S_R1, c, S_C1, 1, 1, 1)  # u32
                else:
                    # last chunk: c0's allreduce fires after u31's drain and
                    # overlaps u32's matmuls+drain; only c1's allreduce trails
                    off_chunk(S_R1, c, S_C0, 1, 0, 0)
                    col_allreduce(0)                   # c0 cols (u01 + u31)
                    off_chunk(S_R1, c, S_C1, 1, 1, 1)
                    # c1 col-max via PE transpose + DVE reduce (PE/DVE are
                    # idle here; avoids 2 serial gpsimd allreduces in the tail)
                    for v in range(V):
                        tpc = trp.tile([128, V, 2, 128], F32, tag="tp", name="tpc")
                        for q in range(4):
                            nc.tensor.transpose(
                                tpc[:, q // 2, q % 2, :],
                                cmx[:, v, 1, 128 * q : 128 * (q + 1)], identF)
                        nc.vector.reduce_max(
                            cmt[:, v, :], tpc[:],
                            axis=mybir.AxisListType.X)

        # ---- skewed pipeline over all 16 chunks ----
        chunks = [(s, c) for s in range(4) for c in range(NCH)]
        for i, (s, c) in enumerate(chunks):
            stage_a(s, c, both_dve=(i % 2 == 1))
            if i == 0:
                load_consts()
            if i > 0:
                ps, pc = chunks[i - 1]
                stage_b(ps, pc, split=(i <= 2))
                gram_for(ps, pc)
        stage_b(*chunks[-1])
        gram_for(*chunks[-1])

        rmfin = const.tile([128, V, 3, NCH], F32, name="rmfin")
        nc.vector.reduce_max(rmfin[:], rmcoll[:], axis=mybir.AxisListType.X)
        nc.sync.dma_start(rm_d, rmfin.rearrange("p v r c -> p (v r c)"))
        nc.sync.dma_start(cm_d, cmr[0:1].rearrange("p v i j -> p (v i j)"))
        nc.sync.dma_start(cmt_d, cmt.rearrange("p v q -> p (v q)"))

    nc.compile()
    return nc


_CACHED = {}


def _consts():
    cstF = np.zeros((128, 129), np.float32)
    cstF[:, 0:128] = np.eye(128)
    cstF[:, 128] = EPS / (YSC * YSC)
    cstB = np.zeros((128, 256), np.float32)
    cstB[:, 0:128] = np.eye(128)
    cstB[:, 128:256] = MASKV * np.eye(128)
    import ml_dtypes
    return cstF, cstB.astype(ml_dtypes.bfloat16)


def _run(x, trace=False):
    x = np.ascontiguousarray(np.asarray(x, dtype=np.float32))
    assert x.shape == (B, V, D), x.shape
    if "nc" not in _CACHED:
        _CACHED["nc"] = build()
    nc = _CACHED["nc"]
    cstF, cstB = _consts()
    xr = x.reshape(G, GS, V, D)
    in_maps = []
    for r0, r1, c0, c1 in CORES:
        in_maps.append({
            "xg": np.ascontiguousarray(xr[[c0, r0, c1, r1]]),
            "cstF": cstF,
            "cstB": cstB,
        })
    res = bass_utils.run_bass_kernel_spmd(
        nc, in_maps, core_ids=list(range(NCORES)), trace=trace)

    M = np.full((V, B), MASKV, np.float32)
    for k, (r0, r1, c0, c1) in enumerate(CORES):
        rm = np.asarray(res.results[k]["rm"]).reshape(128, V, 3, NCH)
        cm = np.asarray(res.results[k]["cm"]).reshape(V, 3, GS)
        cmt = np.asarray(res.results[k]["cmt"]).reshape(128, V, 4)
        for v in range(V):
            for ri, g in ((0, r0), (1, r1), (2, c1)):
                seg = M[v, g * GS : (g + 1) * GS]
                np.maximum(seg, rm[:, v, ri, :].T.reshape(GS), out=seg)
            for ci, g in ((0, c0), (2, r0)):
                seg = M[v, g * GS : (g + 1) * GS]
                np.maximum(seg, cm[v, ci], out=seg)
            seg = M[v, c1 * GS : (c1 + 1) * GS]
            np.maximum(seg, cmt[:, v, :].T.reshape(GS), out=seg)

    m = M.astype(np.float64) / (YSC * YSC)
    dist = np.sqrt(np.maximum(2.0 - 2.0 * m, 0.0))
    total = np.float32(-np.sum(np.mean(np.log(dist + EPS), axis=1)))
    return total, res


def kernel(student_global_cls_tokens):
    total, _ = _run(student_global_cls_tokens, trace=False)
    return np.asarray(total, dtype=np.float32)
